# revision 29
# baseline (speedup 1.0000x reference)
"""GQA attention block (QKV proj + RoPE + KV cache append + softmax attention)
on 8 Trainium2 NeuronCores, tensor-parallel over heads.

Sharding: core c owns q-heads [4c, 4c+4) and kv-head c. Each core computes its
head slice over all tokens; host concatenates the per-core output columns.

start_pos is specialized to 0 (the cache is zero-filled and fully overwritten
by the current 2048 tokens, so keys/values == rope(x@wk), x@wv).
"""

import sys

sys.path.insert(0, "/opt/trn_rl_repo")

import numpy as np

import concourse.bass as bass
import concourse.tile as tile
from concourse import bacc, mybir
from concourse.bass_utils import run_bass_kernel_spmd
from concourse.masks import make_identity

F32 = mybir.dt.float32
BF16 = mybir.dt.bfloat16

B, S, D = 2, 2048, 4096
HQ, HKV, HD = 32, 8, 128
NCORES = 8
HPC = HQ // NCORES          # q heads per core
QDIM = HPC * HD             # per-core q output dim (512)
TOK = B * S                 # 4096 tokens across both batches
KCH = D // 128              # 32 contraction chunks of 128
PCH = 8                     # projection token chunks
PCW = TOK // PCH            # 512 tokens per chunk
SCH = 4                     # s-chunks per batch in attention
SCW = S // SCH              # 512
NTT = S // 128              # 16 key tiles per batch
SCALE = 1.0 / float(np.sqrt(HD))

LAST_EXEC_NS = None


def _build_program():
    nc = bacc.Bacc("TRN2", target_bir_lowering=False, debug=False,
                   num_devices=NCORES)

    xt = nc.declare_dram_parameter("xt", [D, TOK], F32, isOutput=False)
    wq = nc.declare_dram_parameter("wq", [D, QDIM], F32, isOutput=False)
    wk = nc.declare_dram_parameter("wk", [D, HD], F32, isOutput=False)
    wv = nc.declare_dram_parameter("wv", [D, HD], F32, isOutput=False)
    cc = nc.declare_dram_parameter("cc", [128, TOK], F32, isOutput=False)
    ss = nc.declare_dram_parameter("ss", [128, TOK], F32, isOutput=False)
    out = nc.declare_dram_parameter("out", [B, S, QDIM], F32, isOutput=True)

    with tile.TileContext(nc) as tc:
        pers_cm = tc.tile_pool(name="pers", bufs=1)
        pers = pers_cm.__enter__()

        ccs = pers.tile([128, TOK], F32)
        sss = pers.tile([128, TOK], F32)
        qTb = pers.tile([128, HPC, TOK], BF16)   # [d, head, tok]
        kTb = pers.tile([128, TOK], BF16)        # [d, tok]
        vTb = pers.tile([128, TOK], BF16)        # [dv, tok]
        vtok = pers.tile([128, B * NTT, HD], BF16)  # [t, (b,tt), dv]
        id_bf = pers.tile([128, 128], BF16)
        id_f32 = pers.tile([128, 128], F32)
        ones128 = pers.tile([128, 128], BF16)

        nc.sync.dma_start(out=ccs, in_=cc[:])
        nc.sync.dma_start(out=sss, in_=ss[:])
        make_identity(nc, id_bf)
        make_identity(nc, id_f32)
        nc.vector.memset(ones128, 1.0)

        # ---------------- phase 1: projections + rope ----------------
        with tc.tile_pool(name="wpool", bufs=1) as wpool:
            wqb = wpool.tile([128, KCH, QDIM], BF16)
            wkb = wpool.tile([128, KCH, HD], BF16)
            wvb = wpool.tile([128, KCH, HD], BF16)
            # cast-load weights (already column-permuted on host for rope),
            # one DMA per contraction chunk so the first matmuls can start
            # as soon as the kc=0 slices land
            for kc in range(KCH):
                nc.gpsimd.dma_start(
                    out=wqb[:, kc, :], in_=wq[kc * 128:(kc + 1) * 128, :])
                nc.gpsimd.dma_start(
                    out=wkb[:, kc, :], in_=wk[kc * 128:(kc + 1) * 128, :])
                nc.gpsimd.dma_start(
                    out=wvb[:, kc, :], in_=wv[kc * 128:(kc + 1) * 128, :])

            with (
                tc.tile_pool(name="xfp", bufs=6) as xfp,
                tc.tile_pool(name="xTp", bufs=8) as xTp,
                tc.tile_pool(name="pp1", bufs=6, space="PSUM") as pp1,
                tc.tile_pool(name="ropep", bufs=4) as ropep,
            ):
                for pc in range(PCH):
                    tok_sl = bass.ds(pc * PCW, PCW)
                    psums = []
                    for ot in range(6):
                        psums.append(pp1.tile([128, PCW], F32, tag="proj", name="proj"))
                    for kc in range(KCH):
                        xf = xfp.tile([128, PCW], F32, tag="xf", name="xf")
                        nc.sync.dma_start(
                            out=xf,
                            in_=xt[kc * 128:(kc + 1) * 128,
                                   pc * PCW:(pc + 1) * PCW],
                        )
                        xT = xTp.tile([128, PCW], BF16, tag="xT", name="xT")
                        # f32 -> bf16 cast, alternating engines for balance
                        if kc % 2 == 0:
                            nc.scalar.copy(xT, xf)
                        else:
                            nc.vector.tensor_copy(xT, xf)
                        for ot in range(6):
                            if ot < HPC:
                                lhsT = wqb[:, kc, ot * 128:(ot + 1) * 128]
                            elif ot == HPC:
                                lhsT = wkb[:, kc, :]
                            else:
                                lhsT = wvb[:, kc, :]
                            nc.tensor.matmul(
                                psums[ot], lhsT, xT,
                                start=(kc == 0), stop=(kc == KCH - 1),
                            )
                    # epilogues
                    for ot in range(6):
                        ps = psums[ot]
                        if ot < HPC + 1:  # rope for q heads and k
                            t1 = ropep.tile([128, PCW], F32, tag="t1")
                            t2 = ropep.tile([128, PCW], F32, tag="t2")
                            swp = ropep.tile([128, PCW], F32, tag="swp")
                            nc.vector.tensor_mul(t1, ps, ccs[:, tok_sl])
                            # pair-partner swap: cross-partition-base copies
                            # (single-input ops may shift partition windows)
                            nc.scalar.copy(swp[0:64], ps[64:128])
                            nc.scalar.copy(swp[64:128], ps[0:64])
                            nc.vector.tensor_mul(t2, swp, sss[:, tok_sl])
                            if ot < HPC:
                                dst = qTb[:, ot, tok_sl]
                            else:
                                dst = kTb[:, tok_sl]
                            nc.vector.tensor_add(dst, t1, t2)
                        else:
                            nc.scalar.copy(vTb[:, tok_sl], ps)
                    # V to token-major once each batch's chunks are done
                    if pc in (PCH // 2 - 1, PCH - 1):
                        b = 0 if pc < PCH // 2 else 1
                        for tt in range(NTT):
                            pt = pp1.tile([128, 128], BF16, tag="vt",
                                          name="pt", bufs=2)
                            nc.tensor.transpose(
                                pt,
                                vTb[:, b * S + tt * 128:b * S + (tt + 1) * 128],
                                id_bf)
                            nc.vector.tensor_copy(
                                vtok[:, b * NTT + tt, :], pt)

        # ---------------- phase 2: attention ----------------
        with (
            tc.tile_pool(name="psS", bufs=2, space="PSUM") as psS,
            tc.tile_pool(name="psO", bufs=2, space="PSUM") as psO,
            tc.tile_pool(name="psM", bufs=2, space="PSUM") as psM,
            tc.tile_pool(name="expp", bufs=18) as expp,
            tc.tile_pool(name="trep", bufs=6) as trep,
            tc.tile_pool(name="fin", bufs=4) as finp,
        ):
            def attn_head(b, h, sc):
                """scores -> exp -> AV + denominator; returns tail state."""
                q_rhs = qTb[:, h, bass.ds(b * S + sc * SCW, SCW)]
                exps = []
                for g in range(NTT // 2):
                    pS = psS.tile([128, 2 * SCW], F32, tag="S", name="pS")
                    for j in range(2):
                        tt = 2 * g + j
                        nc.tensor.matmul(
                            pS[:, j * SCW:(j + 1) * SCW],
                            kTb[:, b * S + tt * 128:b * S + (tt + 1) * 128],
                            q_rhs, start=True, stop=True,
                        )
                    eS = expp.tile([128, 2 * SCW], BF16, tag="e", name="eS")
                    nc.scalar.activation(
                        out=eS, in_=pS,
                        func=mybir.ActivationFunctionType.Exp,
                        scale=SCALE,
                    )
                    exps.append(eS)
                po = psO.tile([128, SCW], F32, tag="o", name="po")
                for tt in range(NTT):
                    e_rhs = exps[tt // 2][:, (tt % 2) * SCW:
                                          (tt % 2 + 1) * SCW]
                    nc.tensor.matmul(
                        po, vtok[:, b * NTT + tt, :], e_rhs,
                        start=(tt == 0), stop=(tt == NTT - 1),
                    )
                # denominator: per-lane partial sums on DVE (2 tree levels,
                # 16 -> 4 tiles), then 4 all-ones matmuls reduce partitions
                lvl0 = []
                for g in range(NTT // 2):
                    p0 = trep.tile([128, SCW], BF16, tag="tr0", name="p0", bufs=10)
                    nc.vector.tensor_add(
                        p0, exps[g][:, 0:SCW], exps[g][:, SCW:2 * SCW])
                    lvl0.append(p0)
                lvl1 = []
                for g in range(NTT // 4):
                    p1 = trep.tile([128, SCW], BF16, tag="tr1", name="p1", bufs=6)
                    nc.vector.tensor_add(p1, lvl0[2 * g], lvl0[2 * g + 1])
                    lvl1.append(p1)
                pden = psM.tile([128, SCW], F32, tag="m", name="pden")
                for g in range(NTT // 4):
                    nc.tensor.matmul(
                        pden, ones128, lvl1[g],
                        start=(g == 0), stop=(g == NTT // 4 - 1),
                    )
                recip = finp.tile([128, SCW], F32, tag="recip", name="recip")
                nc.vector.reciprocal(recip, pden)
                return (b, h, sc, po, recip)

            def attn_tail(state):
                """normalize -> transpose to token-major -> DMA out.
                Emitted one chunk late so PE rolls straight into the next
                chunk's matmuls instead of waiting on the DVE epilogue."""
                b, h, sc, po, recip = state
                osb = finp.tile([128, SCW], F32, tag="osb", name="osb")
                nc.vector.tensor_mul(osb, po, recip)
                ptr = psM.tile([128, SCW], F32, tag="m", name="ptr")
                for i in range(SCW // 128):
                    nc.tensor.transpose(
                        ptr[:, i * 128:(i + 1) * 128],
                        osb[:, i * 128:(i + 1) * 128],
                        id_f32)
                otok = finp.tile([128, SCW], F32, tag="otok", name="otok")
                nc.vector.tensor_copy(otok, ptr)
                for i in range(SCW // 128):
                    nc.sync.dma_start(
                        out=out[b,
                                sc * SCW + i * 128:sc * SCW + (i + 1) * 128,
                                h * 128:(h + 1) * 128],
                        in_=otok[:, i * 128:(i + 1) * 128],
                    )

            pending = None
            for b in range(B):
                for h in range(HPC):
                    for sc in range(SCH):
                        state = attn_head(b, h, sc)
                        if pending is not None:
                            attn_tail(pending)
                        pending = state
            attn_tail(pending)

        pers_cm.__exit__(None, None, None)

    nc.finalize()
    return nc


_ROPE_PERM = np.concatenate(
    [np.arange(0, HD, 2), np.arange(1, HD, 2)])  # even dims then odd dims


def _shard_inputs(x, wq, wk, wv, freqs_cos, freqs_sin):
    x_flat = np.ascontiguousarray(x.astype(np.float32).reshape(TOK, D))
    xT = np.ascontiguousarray(x_flat.T)                          # [D, TOK]
    cosT = np.ascontiguousarray(freqs_cos.T.astype(np.float32))  # [64, S]
    sinT = np.ascontiguousarray(freqs_sin.T.astype(np.float32))
    cc1 = np.concatenate([cosT, cosT], axis=0)          # [128, S]
    ss1 = np.concatenate([-sinT, sinT], axis=0)         # [128, S]
    cc = np.ascontiguousarray(np.tile(cc1, (1, B)))     # [128, TOK]
    ssm = np.ascontiguousarray(np.tile(ss1, (1, B)))

    in_maps = []
    for c in range(NCORES):
        wq_c = np.empty((D, QDIM), np.float32)
        for j in range(HPC):
            h = HPC * c + j
            wq_c[:, j * HD:(j + 1) * HD] = wq[:, h * HD + _ROPE_PERM]
        wk_c = np.ascontiguousarray(wk[:, c * HD + _ROPE_PERM])
        wv_c = np.ascontiguousarray(wv[:, c * HD:(c + 1) * HD])
        in_maps.append({
            "xt": xT,
            "wq": wq_c, "wk": wk_c, "wv": wv_c,
            "cc": cc, "ss": ssm,
        })
    return in_maps


def kernel(x, wq, wk, wv, cache_k, cache_v, freqs_cos, freqs_sin, start_pos):
    global LAST_EXEC_NS
    x = np.asarray(x)
    wq, wk, wv = np.asarray(wq), np.asarray(wk), np.asarray(wv)
    freqs_cos, freqs_sin = np.asarray(freqs_cos), np.asarray(freqs_sin)
    assert int(start_pos) == 0, "kernel specialized for start_pos == 0"
    assert x.shape == (B, S, D)

    nc = _build_program()
    in_maps = _shard_inputs(x, wq, wk, wv, freqs_cos, freqs_sin)
    res = run_bass_kernel_spmd(nc, in_maps, core_ids=list(range(NCORES)))
    LAST_EXEC_NS = res.exec_time_ns

    full = np.empty((B, S, HQ * HD), np.float32)
    for c in range(NCORES):
        full[:, :, c * QDIM:(c + 1) * QDIM] = res.results[c]["out"]
    return full


# revision 32
# speedup vs baseline: 1.0629x; 1.0629x over previous
"""GQA attention block (QKV proj + RoPE + KV cache append + softmax attention)
on 8 Trainium2 NeuronCores, tensor-parallel over heads.

Sharding: core c owns q-heads [4c, 4c+4) and kv-head c. Each core computes its
head slice over all tokens; host concatenates the per-core output columns.

start_pos is specialized to 0 (the cache is zero-filled and fully overwritten
by the current 2048 tokens, so keys/values == rope(x@wk), x@wv).
"""

import sys

sys.path.insert(0, "/opt/trn_rl_repo")

import numpy as np

import concourse.bass as bass
import concourse.tile as tile
from concourse import bacc, mybir
from concourse.bass_utils import run_bass_kernel_spmd
from concourse.masks import make_identity

F32 = mybir.dt.float32
BF16 = mybir.dt.bfloat16

B, S, D = 2, 2048, 4096
HQ, HKV, HD = 32, 8, 128
NCORES = 8
HPC = HQ // NCORES          # q heads per core
QDIM = HPC * HD             # per-core q output dim (512)
TOK = B * S                 # 4096 tokens across both batches
KCH = D // 128              # 32 contraction chunks of 128
PCH = 8                     # projection token chunks
PCW = TOK // PCH            # 512 tokens per chunk
SCH = 4                     # s-chunks per batch in attention
SCW = S // SCH              # 512
NTT = S // 128              # 16 key tiles per batch
SCALE = 1.0 / float(np.sqrt(HD))

LAST_EXEC_NS = None


def _build_program():
    nc = bacc.Bacc("TRN2", target_bir_lowering=False, debug=False,
                   num_devices=NCORES)

    xt = nc.declare_dram_parameter("xt", [D, TOK], F32, isOutput=False)
    wq = nc.declare_dram_parameter("wq", [D, QDIM], F32, isOutput=False)
    wk = nc.declare_dram_parameter("wk", [D, HD], F32, isOutput=False)
    wv = nc.declare_dram_parameter("wv", [D, HD], F32, isOutput=False)
    cc = nc.declare_dram_parameter("cc", [128, TOK], F32, isOutput=False)
    ss = nc.declare_dram_parameter("ss", [128, TOK], F32, isOutput=False)
    out = nc.declare_dram_parameter("out", [B, S, QDIM], F32, isOutput=True)

    with tile.TileContext(nc) as tc:
        pers_cm = tc.tile_pool(name="pers", bufs=1)
        pers = pers_cm.__enter__()

        ccs = pers.tile([128, TOK], F32)
        sss = pers.tile([128, TOK], F32)
        qTb = pers.tile([128, HPC, TOK], BF16)   # [d, head, tok]
        kTb = pers.tile([128, TOK], BF16)        # [d, tok]
        vTb = pers.tile([128, TOK], BF16)        # [dv, tok]
        vtok = pers.tile([128, B * NTT, HD], BF16)  # [t, (b,tt), dv]
        id_bf = pers.tile([128, 128], BF16)
        id_f32 = pers.tile([128, 128], F32)
        ones128 = pers.tile([128, 128], BF16)

        nc.sync.dma_start(out=ccs, in_=cc[:])
        nc.sync.dma_start(out=sss, in_=ss[:])
        make_identity(nc, id_bf)
        make_identity(nc, id_f32)
        nc.vector.memset(ones128, 1.0)

        # ---------------- phase 1: projections + rope ----------------
        with tc.tile_pool(name="wpool", bufs=1) as wpool:
            wqb = wpool.tile([128, KCH, QDIM], BF16)
            wkb = wpool.tile([128, KCH, HD], BF16)
            wvb = wpool.tile([128, KCH, HD], BF16)
            # cast-load weights (already column-permuted on host for rope),
            # one DMA per contraction chunk so the first matmuls can start
            # as soon as the kc=0 slices land
            for kc in range(KCH):
                nc.gpsimd.dma_start(
                    out=wqb[:, kc, :], in_=wq[kc * 128:(kc + 1) * 128, :])
                nc.gpsimd.dma_start(
                    out=wkb[:, kc, :], in_=wk[kc * 128:(kc + 1) * 128, :])
                nc.gpsimd.dma_start(
                    out=wvb[:, kc, :], in_=wv[kc * 128:(kc + 1) * 128, :])

            with (
                tc.tile_pool(name="xfp", bufs=6) as xfp,
                tc.tile_pool(name="xTp", bufs=8) as xTp,
                tc.tile_pool(name="pp1", bufs=6, space="PSUM") as pp1,
                tc.tile_pool(name="ropep", bufs=4) as ropep,
            ):
                for pc in range(PCH):
                    tok_sl = bass.ds(pc * PCW, PCW)
                    psums = []
                    for ot in range(6):
                        psums.append(pp1.tile([128, PCW], F32, tag="proj", name="proj"))
                    for kc in range(KCH):
                        xf = xfp.tile([128, PCW], F32, tag="xf", name="xf")
                        nc.sync.dma_start(
                            out=xf,
                            in_=xt[kc * 128:(kc + 1) * 128,
                                   pc * PCW:(pc + 1) * PCW],
                        )
                        xT = xTp.tile([128, PCW], BF16, tag="xT", name="xT")
                        # f32 -> bf16 cast, alternating engines for balance
                        if kc % 2 == 0:
                            nc.scalar.copy(xT, xf)
                        else:
                            nc.vector.tensor_copy(xT, xf)
                        for ot in range(6):
                            if ot < HPC:
                                lhsT = wqb[:, kc, ot * 128:(ot + 1) * 128]
                            elif ot == HPC:
                                lhsT = wkb[:, kc, :]
                            else:
                                lhsT = wvb[:, kc, :]
                            nc.tensor.matmul(
                                psums[ot], lhsT, xT,
                                start=(kc == 0), stop=(kc == KCH - 1),
                            )
                    # epilogues
                    for ot in range(6):
                        ps = psums[ot]
                        if ot < HPC + 1:  # rope for q heads and k
                            t1 = ropep.tile([128, PCW], F32, tag="t1")
                            t2 = ropep.tile([128, PCW], F32, tag="t2")
                            swp = ropep.tile([128, PCW], F32, tag="swp")
                            nc.vector.tensor_mul(t1, ps, ccs[:, tok_sl])
                            # pair-partner swap: cross-partition-base copies
                            # (single-input ops may shift partition windows)
                            nc.scalar.copy(swp[0:64], ps[64:128])
                            nc.scalar.copy(swp[64:128], ps[0:64])
                            nc.vector.tensor_mul(t2, swp, sss[:, tok_sl])
                            if ot < HPC:
                                dst = qTb[:, ot, tok_sl]
                            else:
                                dst = kTb[:, tok_sl]
                            nc.vector.tensor_add(dst, t1, t2)
                        else:
                            nc.scalar.copy(vTb[:, tok_sl], ps)
                    # V to token-major once each batch's chunks are done
                    if pc in (PCH // 2 - 1, PCH - 1):
                        b = 0 if pc < PCH // 2 else 1
                        for tt in range(NTT):
                            pt = pp1.tile([128, 128], BF16, tag="vt",
                                          name="pt", bufs=2)
                            nc.tensor.transpose(
                                pt,
                                vTb[:, b * S + tt * 128:b * S + (tt + 1) * 128],
                                id_bf)
                            nc.vector.tensor_copy(
                                vtok[:, b * NTT + tt, :], pt)

        # ---------------- phase 2: attention ----------------
        with (
            tc.tile_pool(name="psS", bufs=2, space="PSUM") as psS,
            tc.tile_pool(name="psO", bufs=2, space="PSUM") as psO,
            tc.tile_pool(name="psM", bufs=2, space="PSUM") as psM,
            tc.tile_pool(name="expp", bufs=26) as expp,
            tc.tile_pool(name="trep", bufs=6) as trep,
            tc.tile_pool(name="fin", bufs=4) as finp,
        ):
            def attn_scores(b, h, sc):
                """scores -> exp (PE + ACT front half of a chunk)."""
                q_rhs = qTb[:, h, bass.ds(b * S + sc * SCW, SCW)]
                exps = []
                for g in range(NTT // 2):
                    pS = psS.tile([128, 2 * SCW], F32, tag="S", name="pS")
                    for j in range(2):
                        tt = 2 * g + j
                        nc.tensor.matmul(
                            pS[:, j * SCW:(j + 1) * SCW],
                            kTb[:, b * S + tt * 128:b * S + (tt + 1) * 128],
                            q_rhs, start=True, stop=True,
                        )
                    eS = expp.tile([128, 2 * SCW], BF16, tag="e", name="eS")
                    nc.scalar.activation(
                        out=eS, in_=pS,
                        func=mybir.ActivationFunctionType.Exp,
                        scale=SCALE,
                    )
                    exps.append(eS)
                return (b, h, sc, exps)

            def attn_av(state):
                """AV matmuls + denominator (emitted one chunk behind the
                scores so the exp pipeline stays ahead of AV consumption)."""
                b, h, sc, exps = state
                po = psO.tile([128, SCW], F32, tag="o", name="po")
                for tt in range(NTT):
                    e_rhs = exps[tt // 2][:, (tt % 2) * SCW:
                                          (tt % 2 + 1) * SCW]
                    nc.tensor.matmul(
                        po, vtok[:, b * NTT + tt, :], e_rhs,
                        start=(tt == 0), stop=(tt == NTT - 1),
                    )
                # denominator: per-lane partial sums on DVE (2 tree levels,
                # 16 -> 4 tiles), then 4 all-ones matmuls reduce partitions
                lvl0 = []
                for g in range(NTT // 2):
                    p0 = trep.tile([128, SCW], BF16, tag="tr0", name="p0",
                                   bufs=10)
                    nc.vector.tensor_add(
                        p0, exps[g][:, 0:SCW], exps[g][:, SCW:2 * SCW])
                    lvl0.append(p0)
                lvl1 = []
                for g in range(NTT // 4):
                    p1 = trep.tile([128, SCW], BF16, tag="tr1", name="p1",
                                   bufs=6)
                    nc.vector.tensor_add(p1, lvl0[2 * g], lvl0[2 * g + 1])
                    lvl1.append(p1)
                pden = psM.tile([128, SCW], F32, tag="m", name="pden")
                for g in range(NTT // 4):
                    nc.tensor.matmul(
                        pden, ones128, lvl1[g],
                        start=(g == 0), stop=(g == NTT // 4 - 1),
                    )
                recip = finp.tile([128, SCW], F32, tag="recip", name="recip")
                nc.vector.reciprocal(recip, pden)
                return (b, h, sc, po, recip)

            def attn_tail(state):
                """normalize -> transpose to token-major -> DMA out.
                Emitted one chunk late so PE rolls straight into the next
                chunk's matmuls instead of waiting on the DVE epilogue."""
                b, h, sc, po, recip = state
                osb = finp.tile([128, SCW], F32, tag="osb", name="osb")
                nc.vector.tensor_mul(osb, po, recip)
                ptr = psM.tile([128, SCW], F32, tag="m", name="ptr")
                for i in range(SCW // 128):
                    nc.tensor.transpose(
                        ptr[:, i * 128:(i + 1) * 128],
                        osb[:, i * 128:(i + 1) * 128],
                        id_f32)
                otok = finp.tile([128, SCW], F32, tag="otok", name="otok")
                nc.vector.tensor_copy(otok, ptr)
                for i in range(SCW // 128):
                    nc.sync.dma_start(
                        out=out[b,
                                sc * SCW + i * 128:sc * SCW + (i + 1) * 128,
                                h * 128:(h + 1) * 128],
                        in_=otok[:, i * 128:(i + 1) * 128],
                    )

            chunks = [(b, h, sc)
                      for b in range(B) for h in range(HPC)
                      for sc in range(SCH)]
            sc_pend = None   # scores emitted, AV not yet
            av_pend = None   # AV emitted, tail not yet
            for key in chunks:
                st = attn_scores(*key)
                if sc_pend is not None:
                    av_pend2 = attn_av(sc_pend)
                    if av_pend is not None:
                        attn_tail(av_pend)
                    av_pend = av_pend2
                sc_pend = st
            av_pend2 = attn_av(sc_pend)
            if av_pend is not None:
                attn_tail(av_pend)
            attn_tail(av_pend2)

        pers_cm.__exit__(None, None, None)

    nc.finalize()
    return nc


_ROPE_PERM = np.concatenate(
    [np.arange(0, HD, 2), np.arange(1, HD, 2)])  # even dims then odd dims


def _shard_inputs(x, wq, wk, wv, freqs_cos, freqs_sin):
    x_flat = np.ascontiguousarray(x.astype(np.float32).reshape(TOK, D))
    xT = np.ascontiguousarray(x_flat.T)                          # [D, TOK]
    cosT = np.ascontiguousarray(freqs_cos.T.astype(np.float32))  # [64, S]
    sinT = np.ascontiguousarray(freqs_sin.T.astype(np.float32))
    cc1 = np.concatenate([cosT, cosT], axis=0)          # [128, S]
    ss1 = np.concatenate([-sinT, sinT], axis=0)         # [128, S]
    cc = np.ascontiguousarray(np.tile(cc1, (1, B)))     # [128, TOK]
    ssm = np.ascontiguousarray(np.tile(ss1, (1, B)))

    in_maps = []
    for c in range(NCORES):
        wq_c = np.empty((D, QDIM), np.float32)
        for j in range(HPC):
            h = HPC * c + j
            wq_c[:, j * HD:(j + 1) * HD] = wq[:, h * HD + _ROPE_PERM]
        wk_c = np.ascontiguousarray(wk[:, c * HD + _ROPE_PERM])
        wv_c = np.ascontiguousarray(wv[:, c * HD:(c + 1) * HD])
        in_maps.append({
            "xt": xT,
            "wq": wq_c, "wk": wk_c, "wv": wv_c,
            "cc": cc, "ss": ssm,
        })
    return in_maps


def kernel(x, wq, wk, wv, cache_k, cache_v, freqs_cos, freqs_sin, start_pos):
    global LAST_EXEC_NS
    x = np.asarray(x)
    wq, wk, wv = np.asarray(wq), np.asarray(wk), np.asarray(wv)
    freqs_cos, freqs_sin = np.asarray(freqs_cos), np.asarray(freqs_sin)
    assert int(start_pos) == 0, "kernel specialized for start_pos == 0"
    assert x.shape == (B, S, D)

    nc = _build_program()
    in_maps = _shard_inputs(x, wq, wk, wv, freqs_cos, freqs_sin)
    res = run_bass_kernel_spmd(nc, in_maps, core_ids=list(range(NCORES)))
    LAST_EXEC_NS = res.exec_time_ns

    full = np.empty((B, S, HQ * HD), np.float32)
    for c in range(NCORES):
        full[:, :, c * QDIM:(c + 1) * QDIM] = res.results[c]["out"]
    return full
